# revision 1
# baseline (speedup 1.0000x reference)
"""CantorGlobalAttention Trainium2 kernel.

Strategy: the routed sparse attention (S=2048, K=64 routes/query, shared
across batch and heads) is computed as DENSE masked attention: the host
builds a multiplicity mask M[j, s] = count of j in routes[s] (so duplicate
route entries and the softmax over route slots are reproduced exactly),
and the device computes
    attn_unnorm = M * exp(scale * (k @ q^T));  out = (v|1)^T @ attn_unnorm
with the denominator obtained from the appended ones column, normalized
afterwards (softmax over 64 slots == masked softmax over all 2048 keys).

Sharding: 8 cores = 2 batches x 4 sequence-quarters. Each core computes
full K/V for its batch (x^T resident), Q only for its 512 query rows, all
8 heads, and the final projection for its rows. No cross-core traffic;
host only slices inputs and concatenates outputs.

Matmuls run in float32r (full-rate fp32, ~1e-4) except the attention-
weights path (exp output / mask / A@V) which is bf16.
"""

import sys

try:
    import concourse.bass as bass  # noqa: F401
except Exception:  # pragma: no cover
    sys.path.insert(0, "/opt/trn_rl_repo")

import numpy as np
import ml_dtypes

import concourse.bass as bass
import concourse.mybir as mybir
import concourse.tile as tile
from concourse.bass_utils import run_bass_kernel_spmd
from concourse.vector_clock import ScopedClock

dt = mybir.dt
AF = mybir.ActivationFunctionType

S = 2048
D = 512
H = 8
HD = 64
B = 2
NCORES = 8
SLICE = 512          # query rows per core
SCALE = HD ** -0.5   # 0.125
NJT = S // 128       # 16 j-tiles
NCHUNK = NJT // 2    # 8 chunks of 2 j-tiles


# ---------------------------------------------------------------------------
# walrus workaround: this walrus build accepts at most ONE sync-wait command
# per instruction; hoist extras onto same-engine nop carriers.
# ---------------------------------------------------------------------------
def _patched_drain_and_barrier(self, tick_clock, wait_clock):
    nc = self.nc
    drain_inst = nc.sync.drain()
    wait_clock.add_sem_waits(
        drain_inst.ins, ScopedClock({None: tick_clock.global_clock})
    )
    nc.all_engine_barrier()
    assert self.sems is not None
    popped = nc._tile_sem_poison_stack.pop()
    assert popped is self._sem_poison
    nc.clear_and_free_semaphores(list(self.sems.allocated().values()))
    nc.all_engine_barrier()


tile.TileContext._drain_and_barrier = _patched_drain_and_barrier


def _split_sync_waits(nc, maxw=1):
    n_fixed = 0
    for fn in nc.m.functions:
        for bb in fn.blocks:
            src = list(bb.instructions)
            out = []
            for inst in src:
                si = inst.sync_info
                waits = list(si.on_wait) if si is not None and si.on_wait else []
                if len(waits) > maxw:
                    keep = waits[-maxw:]
                    carry = waits[:-maxw]
                    for j in range(0, len(carry), maxw):
                        nop = nc.engines[inst.engine].nop(nofuse=True)
                        nc.cur_bb.bb.instructions.remove(nop.ins)
                        nop.ins.sync_info = mybir.SyncInfo(
                            on_wait=list(carry[j : j + maxw]), on_update=[]
                        )
                        out.append(nop.ins)
                    si.on_wait = keep
                    n_fixed += 1
                out.append(inst)
            bb.instructions[:] = out
    return n_fixed


# ---------------------------------------------------------------------------
# device program (identical on all 8 cores; per-core data differs)
# ---------------------------------------------------------------------------
def _build_nc(reps=1, stage=99):
    nc = bass.Bass("TRN2", target_bir_lowering=False, debug=False,
                   num_devices=NCORES)
    f32r, f32, bf16 = dt.float32r, dt.float32, dt.bfloat16

    xT = nc.declare_dram_parameter("xT", [128, 4, S], f32r, isOutput=False)
    xqT = nc.declare_dram_parameter("xqT", [128, 4, SLICE], f32r, isOutput=False)
    wqkvT = nc.declare_dram_parameter("wqkvT", [128, 4, 3 * D], f32r, isOutput=False)
    wprojT = nc.declare_dram_parameter("wprojT", [128, 4, D], f32r, isOutput=False)
    bqkv = nc.declare_dram_parameter("bqkv", [64, 24], f32, isOutput=False)
    bvb = nc.declare_dram_parameter("bvb", [128, D], f32, isOutput=False)
    bprow = nc.declare_dram_parameter("bprow", [1, D], f32r, isOutput=False)
    mt = nc.declare_dram_parameter("mt", [128, NJT, SLICE], bf16, isOutput=False)
    onesr = nc.declare_dram_parameter("onesr", [1, D], f32r, isOutput=False)
    bqr = nc.declare_dram_parameter("bqr", [1, 3 * D], f32r, isOutput=False)
    out = nc.declare_dram_parameter("out", [SLICE, D], f32, isOutput=True)

    with tile.TileContext(nc) as tc:
        with (
            tc.tile_pool(name="const", bufs=1) as constp,
            tc.tile_pool(name="kq", bufs=2) as kqp,
            tc.tile_pool(name="chunk", bufs=6) as chp,
            tc.tile_pool(name="norm", bufs=2) as normp,
            tc.tile_pool(name="psA", bufs=2, space="PSUM") as psA,
            tc.tile_pool(name="psB", bufs=2, space="PSUM") as psB,
            tc.tile_pool(name="dram", bufs=2, space="DRAM") as drp,
        ):
          for rep in range(reps):
            # ---- resident loads: small/hot tensors first, xT split
            #      per D-tile so the first QKV matmuls start early ----
            wq_sb = constp.tile([128, 4, 3 * D], f32r, tag="wqkv")
            nc.sync.dma_start(out=wq_sb[:], in_=wqkvT[:])
            ones_r = constp.tile([1, D], f32r, tag="ones")
            nc.sync.dma_start(out=ones_r[:], in_=onesr[:])
            bqr_sb = constp.tile([1, 3 * D], f32r, tag="bqr")
            nc.sync.dma_start(out=bqr_sb[:], in_=bqr[:])
            bvb_sb = constp.tile([128, D], f32, tag="bvb")
            nc.sync.dma_start(out=bvb_sb[:], in_=bvb[:])
            xt_sb = constp.tile([128, 4, S], f32r, tag="xt")
            for dtile in range(4):
                nc.sync.dma_start(out=xt_sb[:, dtile, :], in_=xT[:, dtile, :])
            xqt_sb = constp.tile([128, 4, SLICE], f32r, tag="xqt")
            nc.sync.dma_start(out=xqt_sb[:], in_=xqT[:])
            mt_sb = constp.tile([128, NJT, SLICE], bf16, tag="mt")
            for half in range(2):
                nc.sync.dma_start(
                    out=mt_sb[:, half * 8 : (half + 1) * 8, :],
                    in_=mt[:, half * 8 : (half + 1) * 8, :],
                )
            wp_sb = constp.tile([128, 4, D], f32r, tag="wproj")
            nc.sync.dma_start(out=wp_sb[:], in_=wprojT[:])
            bq_sb = constp.tile([64, 24], f32, tag="bqkv")
            nc.sync.dma_start(out=bq_sb[:], in_=bqkv[:])
            bp_sb = constp.tile([1, D], f32r, tag="bprow")
            nc.sync.dma_start(out=bp_sb[:], in_=bprow[:])

            # ---- V for all heads, untransposed [j, vdim], bf16, +bias,
            #      with a ones column appended per head (denominator) ----
            v_aug = constp.tile([128, NJT, H * (HD + 1)], bf16, tag="vaug")
            nc.vector.memset(
                v_aug[:, :, :].rearrange("p t (h e) -> p t h e", e=HD + 1)[
                    :, :, :, HD : HD + 1
                ],
                1.0,
            )
            for jt in range(NJT if stage >= 1 else 0):
                vps = psB.tile([128, D], f32, tag="qkvps")
                for dtile in range(4):
                    nc.tensor.matmul(
                        vps[:],
                        xt_sb[:, dtile, jt * 128 : (jt + 1) * 128],
                        wq_sb[:, dtile, 2 * D : 3 * D],
                        start=(dtile == 0),
                        stop=(dtile == 3),
                    )
                dst = v_aug[:, jt, :].rearrange("p (h e) -> p h e", e=HD + 1)[
                    :, :, 0:HD
                ]
                nc.vector.tensor_add(dst, vps[:].rearrange("p (h e) -> p h e", e=HD),
                                     bvb_sb[:].rearrange("p (h e) -> p h e", e=HD))

            # attnout rows (dd = 8*64, pair-stacked) feeding the projection
            ao = [constp.tile([128, SLICE], f32r, tag=f"ao{p}", name=f"ao{p}_{rep}") for p in range(4)]

            # ---- per head-pair: K^T, Q^T, then 2 heads of attention ----
            for pair in range(4 if stage >= 2 else 0):
                kt_e = kqp.tile([64, S], f32r, tag="kte")
                kt_o = kqp.tile([64, S], f32r, tag="kto")
                for jb in range(4):
                    kps = psB.tile([128, D], f32, tag="qkvps")
                    for dtile in range(4):
                        nc.tensor.matmul(
                            kps[:],
                            wq_sb[:, dtile, D + pair * 128 : D + (pair + 1) * 128],
                            xt_sb[:, dtile, jb * 512 : (jb + 1) * 512],
                            start=(dtile == 0),
                            stop=False,
                        )
                    nc.tensor.matmul(
                        kps[:],
                        bqr_sb[:, D + pair * 128 : D + (pair + 1) * 128],
                        ones_r[:],
                        start=False, stop=True,
                    )
                    nc.vector.tensor_copy(
                        kt_e[:, jb * 512 : (jb + 1) * 512], kps[0:64, :])
                    nc.vector.tensor_copy(
                        kt_o[:, jb * 512 : (jb + 1) * 512], kps[64:128, :])
                qt_e = kqp.tile([64, SLICE], f32r, tag="qte")
                qt_o = kqp.tile([64, SLICE], f32r, tag="qto")
                qps = psB.tile([128, SLICE], f32, tag="qkvps")
                for dtile in range(4):
                    nc.tensor.matmul(
                        qps[:],
                        wq_sb[:, dtile, pair * 128 : (pair + 1) * 128],
                        xqt_sb[:, dtile, :],
                        start=(dtile == 0),
                        stop=False,
                    )
                nc.tensor.matmul(
                    qps[:], bqr_sb[:, pair * 128 : (pair + 1) * 128], ones_r[:],
                    start=False, stop=True,
                )
                nc.vector.tensor_copy(qt_e[:], qps[0:64, :])
                nc.vector.tensor_copy(qt_o[:], qps[64:128, :])

                for half, (kt, qt) in enumerate((((kt_e, qt_e), (kt_o, qt_o)) if stage >= 3 else ())):
                    h = 2 * pair + half
                    avps = psB.tile([HD + 1, SLICE], f32, tag="avps")
                    for ch in range(NCHUNK if stage >= 4 else 0):
                        sps = psA.tile([128, 2, SLICE], f32, tag="scores")
                        at = chp.tile([128, 2, SLICE], bf16, tag="at")
                        atm = chp.tile([128, 2, SLICE], bf16, tag="atm")
                        for jc in range(2):
                            jt = 2 * ch + jc
                            nc.tensor.matmul(
                                sps[:, jc, :],
                                kt[:, jt * 128 : (jt + 1) * 128],
                                qt[:],
                                start=True, stop=True,
                            )
                        nc.scalar.activation(at[:], sps[:], AF.Exp, scale=SCALE)
                        nc.vector.tensor_mul(
                            atm[:], at[:], mt_sb[:, 2 * ch : 2 * ch + 2, :]
                        )
                        for jc in range(2):
                            jt = 2 * ch + jc
                            nc.tensor.matmul(
                                avps[:],
                                v_aug[:, jt, h * (HD + 1) : (h + 1) * (HD + 1)],
                                atm[:, jc, :],
                                start=(jt == 0), stop=(jt == NJT - 1),
                            )
                    # normalization: 1/denom via exp(-ln(d)), broadcast via
                    # a DRAM bounce (0-stride partition read), multiply.
                    if stage < 5:
                        continue
                    lnv = normp.tile([1, SLICE], f32, tag="lnv")
                    nc.scalar.activation(lnv[:], avps[64:65, :], AF.Ln)
                    rec = normp.tile([1, SLICE], f32, tag="rec")
                    nc.scalar.activation(rec[:], lnv[:], AF.Exp, scale=-1.0)
                    scr = drp.tile([1, SLICE], f32, tag="scr")
                    nc.sync.dma_start(out=scr[:], in_=rec[:])
                    sap = scr[:]
                    bcast = bass.AP(tensor=sap.tensor, offset=sap.offset,
                                    ap=[[0, 64]] + sap.ap[1:])
                    rsb = normp.tile([64, SLICE], f32, tag="rsb")
                    nc.gpsimd.dma_start(out=rsb[:], in_=bcast)
                    nc.vector.tensor_mul(
                        ao[pair][half * 64 : (half + 1) * 64, :],
                        avps[0:64, :], rsb[:],
                    )

            # ---- projection: out[s, :] = ao_aug^T @ wprojT + bias ----
            for st in range(4 if stage >= 6 else 0):
                ops = psB.tile([128, D], f32, tag="qkvps")
                for kt_i in range(4):
                    nc.tensor.matmul(
                        ops[:],
                        ao[kt_i][:, st * 128 : (st + 1) * 128],
                        wp_sb[:, kt_i, :],
                        start=(kt_i == 0), stop=False,
                    )
                nc.tensor.matmul(
                    ops[:], ones_r[:, 0:128], bp_sb[:],
                    start=False, stop=True,
                )
                osb = normp.tile([128, D], f32, tag="osb")
                nc.vector.tensor_copy(osb[:], ops[:])
                nc.sync.dma_start(out=out[st * 128 : (st + 1) * 128, :], in_=osb[:])

    _split_sync_waits(nc)
    return nc


_NC_CACHE = {}


def _get_nc(reps=1, stage=99):
    if (reps, stage) not in _NC_CACHE:
        _NC_CACHE[(reps, stage)] = _build_nc(reps, stage)
    return _NC_CACHE[(reps, stage)]


# ---------------------------------------------------------------------------
# host wrapper
# ---------------------------------------------------------------------------
def _prep_inputs(x, routes, w_qkv, b_qkv, w_proj, b_proj):
    x = np.asarray(x, dtype=np.float32)
    routes = np.asarray(routes)
    w_qkv = np.asarray(w_qkv, dtype=np.float32)
    b_qkv = np.asarray(b_qkv, dtype=np.float32)
    w_proj = np.asarray(w_proj, dtype=np.float32)
    b_proj = np.asarray(b_proj, dtype=np.float32)

    r = np.clip(routes[:S].astype(np.int64), 0, S - 1)
    # multiplicity mask M[s, j] = count of j in routes[s]
    flat = (np.arange(S, dtype=np.int64)[:, None] * S + r).ravel()
    M = np.bincount(flat, minlength=S * S).reshape(S, S).astype(np.float32)

    # [p, t, n] = w[n, t*128+p] layouts
    def t_layout(w, n_out):  # w: (n_out, 512) -> (128, 4, n_out)
        return np.ascontiguousarray(w.T.reshape(4, 128, n_out).transpose(1, 0, 2))

    wqkvT = t_layout(w_qkv, 3 * D)
    wprojT = t_layout(w_proj, D)
    bq = np.ascontiguousarray(b_qkv.reshape(24, 64).T)       # (64, 24)
    bvb = np.ascontiguousarray(np.tile(b_qkv[2 * D :], (128, 1)))  # (128, 512)
    bprow = np.ascontiguousarray(b_proj[None, :])            # (1, 512)

    in_maps = []
    for c in range(NCORES):
        b = c // 4
        s0 = (c % 4) * SLICE
        xb = x[b]                                            # (S, D)
        xTc = np.ascontiguousarray(xb.T.reshape(4, 128, S).transpose(1, 0, 2))
        xqTc = np.ascontiguousarray(
            xb[s0 : s0 + SLICE].T.reshape(4, 128, SLICE).transpose(1, 0, 2)
        )
        # mt[p, t, s] = M[s0+s, t*128+p]
        mtc = M[s0 : s0 + SLICE].T.reshape(NJT, 128, SLICE).transpose(1, 0, 2)
        mtc = np.ascontiguousarray(mtc.astype(ml_dtypes.bfloat16))
        in_maps.append(
            {
                "xT": xTc, "xqT": xqTc, "wqkvT": wqkvT, "wprojT": wprojT,
                "bqkv": bq, "bvb": bvb, "bprow": bprow, "mt": mtc,
                "onesr": np.ones((1, 512), dtype=np.float32),
                "bqr": np.ascontiguousarray(b_qkv[None, :]),
            }
        )
    return in_maps


def run_cores(in_maps, reps=1, stage=99, **kwargs):
    nc = _get_nc(reps, stage)
    return run_bass_kernel_spmd(nc, in_maps, list(range(NCORES)), **kwargs)


def kernel(x, routes, w_qkv, b_qkv, w_proj, b_proj):
    in_maps = _prep_inputs(x, routes, w_qkv, b_qkv, w_proj, b_proj)
    res = run_cores(in_maps)
    out = np.empty((B, S, D), dtype=np.float32)
    for c in range(NCORES):
        b = c // 4
        s0 = (c % 4) * SLICE
        out[b, s0 : s0 + SLICE] = res.results[c]["out"]
    return out



# revision 3
# speedup vs baseline: 1.1090x; 1.1090x over previous
"""CantorGlobalAttention Trainium2 kernel, v2.

Dense-masked routed attention (multiplicity mask M reproduces the softmax
over 64 route slots exactly), restructured vs v1:

- Hybrid sharding: 8 cores = 2 batches x 2 head-halves x 2 seq-halves.
  Each core: QKV projection for its 4 heads only, dense masked attention
  for its 1024 queries, and a PARTIAL output projection (contraction over
  its 4 heads' 256 ao-dims). Host sums the two head-half partials + b_proj.
- Transposed AV: attn weights (atm, [j, q] layout) are the matmul
  STATIONARY operand, V the moving operand, so the AV output lands as
  [q, head_dim+1] using all 128 partitions (half the PE rows of v1), and
  the softmax denominator (ones column) becomes a per-partition scalar:
  normalization is a DVE reciprocal + per-partition-scale multiply
  (no ln/exp on Act, no DRAM-bounce broadcast).
- Biases for K/Q folded into the PSUM->SBUF copies (per-partition
  tensor_scalar add on the gpsimd/Pool engine).
- bf16 storage for x, weights, mask, K/Q/V, attn weights, ao.
- ao is PE-transposed (identity matmul) to feed the output projection.
"""

import sys

try:
    import concourse.bass as bass  # noqa: F401
except Exception:  # pragma: no cover
    sys.path.insert(0, "/opt/trn_rl_repo")

import numpy as np
import ml_dtypes

import concourse.bass as bass
import concourse.mybir as mybir
import concourse.tile as tile
from concourse.bass_utils import run_bass_kernel_spmd
from concourse.vector_clock import ScopedClock

dt = mybir.dt
AF = mybir.ActivationFunctionType
ALU = mybir.AluOpType

S = 2048
D = 512
H = 8
HD = 64
B = 2
NCORES = 8
QS = 1024            # queries per core
HH = 4               # heads per core
SCALE = HD ** -0.5   # 0.125
NJT = S // 128       # 16 j-tiles
NCHUNK = NJT // 2    # 8 chunks of 2 j-tiles


# ---------------------------------------------------------------------------
# walrus workaround: this walrus build accepts at most ONE sync-wait command
# per instruction; hoist extras onto same-engine nop carriers.
# ---------------------------------------------------------------------------
def _patched_drain_and_barrier(self, tick_clock, wait_clock):
    nc = self.nc
    drain_inst = nc.sync.drain()
    wait_clock.add_sem_waits(
        drain_inst.ins, ScopedClock({None: tick_clock.global_clock})
    )
    nc.all_engine_barrier()
    assert self.sems is not None
    popped = nc._tile_sem_poison_stack.pop()
    assert popped is self._sem_poison
    nc.clear_and_free_semaphores(list(self.sems.allocated().values()))
    nc.all_engine_barrier()


tile.TileContext._drain_and_barrier = _patched_drain_and_barrier


def _split_sync_waits(nc, maxw=1):
    n_fixed = 0
    for fn in nc.m.functions:
        for bb in fn.blocks:
            src = list(bb.instructions)
            out = []
            for inst in src:
                si = inst.sync_info
                waits = list(si.on_wait) if si is not None and si.on_wait else []
                if len(waits) > maxw:
                    keep = waits[-maxw:]
                    carry = waits[:-maxw]
                    for j in range(0, len(carry), maxw):
                        nop = nc.engines[inst.engine].nop(nofuse=True)
                        nc.cur_bb.bb.instructions.remove(nop.ins)
                        nop.ins.sync_info = mybir.SyncInfo(
                            on_wait=list(carry[j : j + maxw]), on_update=[]
                        )
                        out.append(nop.ins)
                    si.on_wait = keep
                    n_fixed += 1
                out.append(inst)
            bb.instructions[:] = out
    return n_fixed


# ---------------------------------------------------------------------------
# device program (identical on all 8 cores; per-core data differs)
# ---------------------------------------------------------------------------
def _build_nc(reps=1, stage=99):
    nc = bass.Bass("TRN2", target_bir_lowering=False, debug=False,
                   num_devices=NCORES)
    f32, bf16 = dt.float32, dt.bfloat16

    xT = nc.declare_dram_parameter("xT", [128, 4, S], bf16, isOutput=False)
    xqT = nc.declare_dram_parameter("xqT", [128, 4, QS], bf16, isOutput=False)
    wqkvh = nc.declare_dram_parameter("wqkvh", [128, 4, 768], bf16, isOutput=False)
    wph = nc.declare_dram_parameter("wph", [128, 2, D], bf16, isOutput=False)
    bqh = nc.declare_dram_parameter("bqh", [128, 2], f32, isOutput=False)
    bkh = nc.declare_dram_parameter("bkh", [128, 2], f32, isOutput=False)
    bvb = nc.declare_dram_parameter("bvb", [128, 256], f32, isOutput=False)
    mt = nc.declare_dram_parameter("mt", [128, NJT, QS], bf16, isOutput=False)
    ident = nc.declare_dram_parameter("ident", [128, 128], bf16, isOutput=False)
    out = nc.declare_dram_parameter("out", [QS, D], bf16, isOutput=True)

    with tile.TileContext(nc) as tc:
        with (
            tc.tile_pool(name="const", bufs=1) as constp,
            tc.tile_pool(name="kqp", bufs=2) as kqp,
            tc.tile_pool(name="mtp", bufs=2) as mtp,
            tc.tile_pool(name="vaugp", bufs=2) as vaugp,
            tc.tile_pool(name="chunk", bufs=3) as chp,
            tc.tile_pool(name="atm", bufs=3) as atmp,
            tc.tile_pool(name="norm", bufs=2) as normp,
            tc.tile_pool(name="ao", bufs=2) as aop,
            tc.tile_pool(name="psS", bufs=2, space="PSUM") as psS,
            tc.tile_pool(name="psV", bufs=1, space="PSUM") as psV,
            tc.tile_pool(name="psT", bufs=1, space="PSUM") as psTp,
            tc.tile_pool(name="psP", bufs=2, space="PSUM") as psP,
        ):
          pending_tail = []
          for rep in range(reps):
            # ---- resident loads ----
            wq_sb = constp.tile([128, 4, 768], bf16, tag="wqkv", name="wq_sb")
            nc.sync.dma_start(out=wq_sb[:], in_=wqkvh[:])
            bq_sb = constp.tile([128, 2], f32, tag="bq", name="bq_sb")
            nc.sync.dma_start(out=bq_sb[:], in_=bqh[:])
            bk_sb = constp.tile([128, 2], f32, tag="bk", name="bk_sb")
            nc.sync.dma_start(out=bk_sb[:], in_=bkh[:])
            bvb_sb = constp.tile([128, 256], f32, tag="bvb", name="bvb_sb")
            nc.sync.dma_start(out=bvb_sb[:], in_=bvb[:])
            id_sb = constp.tile([128, 128], bf16, tag="ident", name="id_sb")
            nc.sync.dma_start(out=id_sb[:], in_=ident[:])
            xt_sb = constp.tile([128, 4, S], bf16, tag="xt", name="xt_sb")
            for jb in range(4):
                nc.sync.dma_start(out=xt_sb[:, :, jb * 512 : (jb + 1) * 512],
                                  in_=xT[:, :, jb * 512 : (jb + 1) * 512])
            xqt_sb = constp.tile([128, 4, QS], bf16, tag="xqt", name="xqt_sb")
            nc.sync.dma_start(out=xqt_sb[:], in_=xqT[:])
            mt_sb = mtp.tile([128, NJT, QS], bf16, tag="mt")
            for piece in range(4):
                nc.sync.dma_start(
                    out=mt_sb[:, piece * 4 : (piece + 1) * 4, :],
                    in_=mt[:, piece * 4 : (piece + 1) * 4, :],
                )
            wp_sb = constp.tile([128, 2, D], bf16, tag="wp", name="wp_sb")
            nc.sync.dma_start(out=wp_sb[:], in_=wph[:])

            # persistent K^T / Q^T (2 head-pairs stacked on partitions), bf16
            kt2 = kqp.tile([128, 2, S], bf16, tag="kt2", name=f"kt2_{rep}")
            qt2 = kqp.tile([128, 2, QS], bf16, tag="qt2", name=f"qt2_{rep}")

            # ---- projection helpers (emitted interleaved below) ----
            def emit_kproj(kp, jb):
                kps = psP.tile([128, 512], f32, tag="pp",
                               name=f"kps_{rep}_{kp}_{jb}")
                for dtile in range(4):
                    nc.tensor.matmul(
                        kps[:],
                        wq_sb[:, dtile, 256 + kp * 128 : 256 + (kp + 1) * 128],
                        xt_sb[:, dtile, jb * 512 : (jb + 1) * 512],
                        start=(dtile == 0),
                        stop=(dtile == 3),
                    )
                nc.vector.tensor_scalar(
                    kt2[:, kp, jb * 512 : (jb + 1) * 512], kps[:],
                    bk_sb[:, kp : kp + 1], None, op0=ALU.add,
                )

            def emit_qproj(kp, qc):
                qps = psP.tile([128, 512], f32, tag="pp",
                               name=f"qps_{rep}_{kp}_{qc}")
                for dtile in range(4):
                    nc.tensor.matmul(
                        qps[:],
                        wq_sb[:, dtile, kp * 128 : (kp + 1) * 128],
                        xqt_sb[:, dtile, qc * 512 : (qc + 1) * 512],
                        start=(dtile == 0),
                        stop=(dtile == 3),
                    )
                nc.vector.tensor_scalar(
                    qt2[:, kp, qc * 512 : (qc + 1) * 512], qps[:],
                    bq_sb[:, kp : kp + 1], None, op0=ALU.add,
                )

            v_aug = vaugp.tile([128, NJT, HH * (HD + 1)], bf16, tag="vaug",
                               name=f"vaug_{rep}")
            nc.vector.memset(
                v_aug[:, :, :].rearrange("p t (h e) -> p t h e", e=HD + 1)[
                    :, :, :, HD : HD + 1
                ],
                1.0,
            )

            def emit_vproj(jt):
                vps = psP.tile([128, 256], f32, tag="pp",
                               name=f"vps_{rep}_{jt}")
                for dtile in range(4):
                    nc.tensor.matmul(
                        vps[:],
                        xt_sb[:, dtile, jt * 128 : (jt + 1) * 128],
                        wq_sb[:, dtile, 512:768],
                        start=(dtile == 0),
                        stop=(dtile == 3),
                    )
                dst = v_aug[:, jt, :].rearrange("p (h e) -> p h e", e=HD + 1)[
                    :, :, 0:HD
                ]
                nc.vector.tensor_add(
                    dst,
                    vps[:].rearrange("p (h e) -> p h e", e=HD),
                    bvb_sb[:].rearrange("p (h e) -> p h e", e=HD),
                )

            # ---- attention units, software-pipelined ----
            # unit u = (qh, hl). emit order: chunks(u) ... AV(u-1), norm(u-1)
            units = [(qh, hl) for qh in range(2) for hl in range(HH)]
            ao_tiles = {}
            unit_state = {}

            def emit_chunks(u, kt2=kt2, qt2=qt2, mt_sb=mt_sb,
                            unit_state=unit_state, rep=rep):
                qh, hl = units[u]
                kp, hp = hl // 2, hl % 2
                ktv = kt2[64 * hp : 64 * hp + 64, kp, :]
                qtv = qt2[64 * hp : 64 * hp + 64, kp,
                          qh * 512 : (qh + 1) * 512]
                atm = atmp.tile([128, NJT, 512], bf16, tag="atm",
                                name=f"atm_{rep}_{u}")
                for ch in range(NCHUNK):
                    sps = psS.tile([128, 2, 512], f32, tag="sc",
                                   name=f"sps_{rep}_{u}_{ch}")
                    at = chp.tile([128, 2, 512], bf16, tag="at",
                                  name=f"at_{rep}_{u}_{ch}")
                    for jc in range(2):
                        jt = 2 * ch + jc
                        nc.tensor.matmul(
                            sps[:, jc, :],
                            ktv[:, jt * 128 : (jt + 1) * 128],
                            qtv[:],
                            start=True, stop=True,
                        )
                    # prologue interleave: V/K/Q projections ride between the
                    # early units' score chunks. AV runs at pipeline depth 2
                    # for the first units, so v_aug is needed only at AV(0)
                    # (emitted after chunks(2)); K1-jb0/Q1-qc0 before unit 2's
                    # first scores, later K1 j-blocks two chunks ahead of use.
                    if u == 0 and stage >= 2:
                        emit_vproj(ch)
                    if u == 1:
                        if stage >= 2:
                            emit_vproj(8 + ch)
                        if stage >= 1 and ch == 3:
                            emit_kproj(1, 0)
                        if stage >= 1 and ch == 5:
                            emit_qproj(1, 0)
                    if u == 2 and stage >= 1:
                        if ch in (0, 2, 4):
                            emit_kproj(1, 1 + ch // 2)
                        elif ch == 6:
                            emit_qproj(0, 1)
                    if u == 3 and ch == 0 and stage >= 1:
                        emit_qproj(1, 1)
                    nc.scalar.activation(at[:], sps[:], AF.Exp, scale=SCALE)
                    nc.vector.tensor_mul(
                        atm[:, 2 * ch : 2 * ch + 2, :], at[:],
                        mt_sb[:, 2 * ch : 2 * ch + 2,
                              qh * 512 : (qh + 1) * 512],
                    )
                unit_state[u] = atm

            def emit_av_norm(u, unit_state=unit_state,
                             ao_tiles=ao_tiles, v_aug=v_aug, rep=rep):
                qh, hl = units[u]
                atm = unit_state[u]
                avps = psV.tile([128, 4, HD + 1], f32, tag="av",
                                name=f"avps_{rep}_{u}")
                for qt in range(4):
                    for jt in range(NJT):
                        nc.tensor.matmul(
                            avps[:, qt, :],
                            atm[:, jt, qt * 128 : (qt + 1) * 128],
                            v_aug[:, jt, hl * (HD + 1) : (hl + 1) * (HD + 1)],
                            start=(jt == 0), stop=(jt == NJT - 1),
                        )
                if stage < 4:
                    return
                ao_sb = ao_tiles[qh]
                for qt in range(4):
                    rec = normp.tile([128, 1], f32, tag="rec",
                                     name=f"rec_{rep}_{u}_{qt}")
                    nc.vector.reciprocal(rec[:], avps[:, qt, HD : HD + 1])
                    nc.vector.tensor_scalar(
                        ao_sb[:, qt, hl * 64 : (hl + 1) * 64],
                        avps[:, qt, 0:HD], rec[:, 0:1], None, op0=ALU.mult,
                    )

            def emit_epilogue(qh, ao_tiles=ao_tiles, id_sb=id_sb,
                              wp_sb=wp_sb, rep=rep):
                # transpose ao + partial output projection for one seq-half
                if stage < 5:
                    return
                ao_sb = ao_tiles[qh]
                psT = psTp.tile([128, 4, 256], bf16, tag="tr",
                                name=f"psT_{rep}_{qh}")
                for st in range(4):
                    for ddt in range(2):
                        nc.tensor.transpose(
                            psT[:, st, ddt * 128 : (ddt + 1) * 128],
                            ao_sb[:, st, ddt * 128 : (ddt + 1) * 128],
                            id_sb[:],
                        )
                aot = normp.tile([128, 4, 256], bf16, tag="aoT",
                                 name=f"aoT_{rep}_{qh}")
                nc.vector.tensor_copy(aot[:], psT[:])
                for st in range(4):
                    ops = psP.tile([128, 512], f32, tag="pp",
                                   name=f"ops_{rep}_{qh}_{st}")
                    for ddt in range(2):
                        nc.tensor.matmul(
                            ops[:], aot[:, st, ddt * 128 : (ddt + 1) * 128],
                            wp_sb[:, ddt, :],
                            start=(ddt == 0), stop=(ddt == 1),
                        )
                    osb = normp.tile([128, D], bf16, tag="osb",
                                     name=f"osb_{rep}_{qh}_{st}")
                    nc.vector.tensor_copy(osb[:], ops[:])
                    nc.gpsimd.dma_start(
                        out=out[qh * 512 + st * 128 : qh * 512 + (st + 1) * 128, :],
                        in_=osb[:],
                    )

            if stage >= 1:
                for jb in range(4):
                    emit_kproj(0, jb)
                emit_qproj(0, 0)
            if stage >= 3:
                ao_tiles[0] = aop.tile([128, 4, 256], bf16, tag="ao",
                                       name=f"ao0_{rep}")
                ao_tiles[1] = aop.tile([128, 4, 256], bf16, tag="ao",
                                       name=f"ao1_{rep}")
                # deferred tail of the previous rep: its last unit's chunks
                # ride in this rep's head windows (this rep's K0/Q0 were just
                # emitted and execute in the previous unit-6 window's slack)
                if pending_tail:
                    t_ch7, t_av7, t_ep1 = pending_tail.pop()
                else:
                    t_ch7 = t_av7 = t_ep1 = None
                if t_ch7 is not None:
                    t_ch7()
                emit_chunks(0)
                if t_av7 is not None:
                    t_av7()
                emit_chunks(1)
                emit_chunks(2)
                emit_av_norm(0)
                emit_chunks(3)
                emit_av_norm(1)
                if t_ep1 is not None:
                    t_ep1()
                emit_chunks(4)
                emit_av_norm(2)
                emit_av_norm(3)
                emit_chunks(5)
                emit_av_norm(4)
                emit_epilogue(0)
                emit_chunks(6)
                emit_av_norm(5)

                def _t_ch7(c=emit_chunks, a=emit_av_norm):
                    c(7)
                    a(6)

                def _t_av7(a=emit_av_norm):
                    a(7)

                def _t_ep1(e=emit_epilogue):
                    e(1)

                pending_tail.append((_t_ch7, _t_av7, _t_ep1))
          if pending_tail:
              for f in pending_tail.pop():
                  f()

    _split_sync_waits(nc)
    return nc


_NC_CACHE = {}


def _get_nc(reps=1, stage=99):
    if (reps, stage) not in _NC_CACHE:
        _NC_CACHE[(reps, stage)] = _build_nc(reps, stage)
    return _NC_CACHE[(reps, stage)]


# ---------------------------------------------------------------------------
# host wrapper
# ---------------------------------------------------------------------------
def _prep_inputs(x, routes, w_qkv, b_qkv, w_proj, b_proj):
    x = np.asarray(x, dtype=np.float32)
    routes = np.asarray(routes)
    w_qkv = np.asarray(w_qkv, dtype=np.float32)
    b_qkv = np.asarray(b_qkv, dtype=np.float32)
    w_proj = np.asarray(w_proj, dtype=np.float32)
    b_proj = np.asarray(b_proj, dtype=np.float32)
    bf = ml_dtypes.bfloat16

    r = np.clip(routes[:S].astype(np.int64), 0, S - 1)
    # multiplicity mask M[s, j] = count of j in routes[s]
    flat = (np.arange(S, dtype=np.int64)[:, None] * S + r).ravel()
    M = np.bincount(flat, minlength=S * S).reshape(S, S).astype(np.float32)

    def t_layout(w, n_out, ntile):  # w: (n_out, 512) -> (128, ntile, n_out)
        return np.ascontiguousarray(
            w.T.reshape(ntile, 128, n_out).transpose(1, 0, 2).astype(bf)
        )

    id128 = np.eye(128, dtype=np.float32).astype(bf)

    in_maps = []
    for c in range(NCORES):
        b, hh, sh = c // 4, (c % 4) // 2, c % 2
        hd0 = hh * 256
        # weights for this head-half: Q rows [hd0, hd0+256), K rows
        # [D+hd0, ...), V rows [2D+hd0, ...) -> [128, 4, 768] (q|k|v)
        wsel = np.concatenate(
            [w_qkv[hd0 : hd0 + 256],
             w_qkv[D + hd0 : D + hd0 + 256],
             w_qkv[2 * D + hd0 : 2 * D + hd0 + 256]], axis=0
        )  # (768, 512)
        wqkvh = t_layout(wsel, 768, 4)
        # wproj columns for this head-half's ao dims -> [128, 2, 512]
        wph = np.ascontiguousarray(
            w_proj[:, hd0 : hd0 + 256].T.reshape(2, 128, D)
            .transpose(1, 0, 2).astype(bf)
        )
        bqh = np.ascontiguousarray(
            b_qkv[hd0 : hd0 + 256].reshape(2, 128).T.astype(np.float32))
        bkh = np.ascontiguousarray(
            b_qkv[D + hd0 : D + hd0 + 256].reshape(2, 128).T.astype(np.float32))
        bvbh = np.ascontiguousarray(
            np.tile(b_qkv[2 * D + hd0 : 2 * D + hd0 + 256], (128, 1))
        ).astype(np.float32)

        xb = x[b]  # (S, D)
        xTc = np.ascontiguousarray(
            xb.T.reshape(4, 128, S).transpose(1, 0, 2).astype(bf))
        s0 = sh * QS
        xqTc = np.ascontiguousarray(
            xb[s0 : s0 + QS].T.reshape(4, 128, QS).transpose(1, 0, 2).astype(bf)
        )
        # mt[p, t, q] = M[s0+q, t*128+p]
        mtc = M[s0 : s0 + QS].T.reshape(NJT, 128, QS).transpose(1, 0, 2)
        mtc = np.ascontiguousarray(mtc.astype(bf))
        in_maps.append(
            {
                "xT": xTc, "xqT": xqTc, "wqkvh": wqkvh, "wph": wph,
                "bqh": bqh, "bkh": bkh, "bvb": bvbh, "mt": mtc,
                "ident": id128,
            }
        )
    return in_maps


def run_cores(in_maps, reps=1, stage=99, **kwargs):
    nc = _get_nc(reps, stage)
    return run_bass_kernel_spmd(nc, in_maps, list(range(NCORES)), **kwargs)


def kernel(x, routes, w_qkv, b_qkv, w_proj, b_proj):
    b_proj = np.asarray(b_proj, dtype=np.float32)
    in_maps = _prep_inputs(x, routes, w_qkv, b_qkv, w_proj, b_proj)
    res = run_cores(in_maps)
    out = np.empty((B, S, D), dtype=np.float32)
    for b in range(B):
        for sh in range(2):
            s0 = sh * QS
            out[b, s0 : s0 + QS] = (
                res.results[b * 4 + sh]["out"].astype(np.float32)
                + res.results[b * 4 + 2 + sh]["out"].astype(np.float32)
                + b_proj
            )
    return out


# revision 4
# speedup vs baseline: 4.3351x; 3.9089x over previous
"""CantorGlobalAttention Trainium2 kernel, v2.

Dense-masked routed attention (multiplicity mask M reproduces the softmax
over 64 route slots exactly), restructured vs v1:

- Hybrid sharding: 8 cores = 2 batches x 2 head-halves x 2 seq-halves.
  Each core: QKV projection for its 4 heads only, dense masked attention
  for its 1024 queries, and a PARTIAL output projection (contraction over
  its 4 heads' 256 ao-dims). Host sums the two head-half partials + b_proj.
- Transposed AV: attn weights (atm, [j, q] layout) are the matmul
  STATIONARY operand, V the moving operand, so the AV output lands as
  [q, head_dim+1] using all 128 partitions (half the PE rows of v1), and
  the softmax denominator (ones column) becomes a per-partition scalar:
  normalization is a DVE reciprocal + per-partition-scale multiply
  (no ln/exp on Act, no DRAM-bounce broadcast).
- Biases for K/Q folded into the PSUM->SBUF copies (per-partition
  tensor_scalar add on the gpsimd/Pool engine).
- bf16 storage for x, weights, mask, K/Q/V, attn weights, ao.
- ao is PE-transposed (identity matmul) to feed the output projection.
"""

import sys

try:
    import concourse.bass as bass  # noqa: F401
except Exception:  # pragma: no cover
    sys.path.insert(0, "/opt/trn_rl_repo")

import numpy as np
import ml_dtypes

import concourse.bass as bass
import concourse.mybir as mybir
import concourse.tile as tile
from concourse.bass_utils import run_bass_kernel_spmd
from concourse.vector_clock import ScopedClock

dt = mybir.dt
AF = mybir.ActivationFunctionType
ALU = mybir.AluOpType

S = 2048
D = 512
H = 8
HD = 64
B = 2
NCORES = 8
QS = 1024            # queries per core
HH = 4               # heads per core
SCALE = HD ** -0.5   # 0.125
NJT = S // 128       # 16 j-tiles
NCHUNK = NJT // 2    # 8 chunks of 2 j-tiles


# ---------------------------------------------------------------------------
# walrus workaround: this walrus build accepts at most ONE sync-wait command
# per instruction; hoist extras onto same-engine nop carriers.
# ---------------------------------------------------------------------------
def _patched_drain_and_barrier(self, tick_clock, wait_clock):
    nc = self.nc
    drain_inst = nc.sync.drain()
    wait_clock.add_sem_waits(
        drain_inst.ins, ScopedClock({None: tick_clock.global_clock})
    )
    nc.all_engine_barrier()
    assert self.sems is not None
    popped = nc._tile_sem_poison_stack.pop()
    assert popped is self._sem_poison
    nc.clear_and_free_semaphores(list(self.sems.allocated().values()))
    nc.all_engine_barrier()


tile.TileContext._drain_and_barrier = _patched_drain_and_barrier


def _split_sync_waits(nc, maxw=1):
    n_fixed = 0
    for fn in nc.m.functions:
        for bb in fn.blocks:
            src = list(bb.instructions)
            out = []
            for inst in src:
                si = inst.sync_info
                waits = list(si.on_wait) if si is not None and si.on_wait else []
                if len(waits) > maxw:
                    keep = waits[-maxw:]
                    carry = waits[:-maxw]
                    for j in range(0, len(carry), maxw):
                        nop = nc.engines[inst.engine].nop(nofuse=True)
                        nc.cur_bb.bb.instructions.remove(nop.ins)
                        nop.ins.sync_info = mybir.SyncInfo(
                            on_wait=list(carry[j : j + maxw]), on_update=[]
                        )
                        out.append(nop.ins)
                    si.on_wait = keep
                    n_fixed += 1
                out.append(inst)
            bb.instructions[:] = out
    return n_fixed


# ---------------------------------------------------------------------------
# device program (identical on all 8 cores; per-core data differs)
# ---------------------------------------------------------------------------
def _build_nc(reps=1, stage=99):
    nc = bass.Bass("TRN2", target_bir_lowering=False, debug=False,
                   num_devices=NCORES)
    f32, bf16 = dt.float32, dt.bfloat16

    xT = nc.declare_dram_parameter("xT", [128, 4, S], bf16, isOutput=False)
    xqT = nc.declare_dram_parameter("xqT", [128, 4, QS], bf16, isOutput=False)
    wqkvh = nc.declare_dram_parameter("wqkvh", [128, 4, 768], bf16, isOutput=False)
    wph = nc.declare_dram_parameter("wph", [128, 2, D], bf16, isOutput=False)
    bqh = nc.declare_dram_parameter("bqh", [128, 2], f32, isOutput=False)
    bkh = nc.declare_dram_parameter("bkh", [128, 2], f32, isOutput=False)
    bvb = nc.declare_dram_parameter("bvb", [128, 256], f32, isOutput=False)
    mt = nc.declare_dram_parameter("mt", [128, NJT, QS], bf16, isOutput=False)
    ident = nc.declare_dram_parameter("ident", [128, 128], bf16, isOutput=False)
    out = nc.declare_dram_parameter("out", [QS, D], bf16, isOutput=True)

    with tile.TileContext(nc) as tc:
        with (
            tc.tile_pool(name="const", bufs=1) as constp,
            tc.tile_pool(name="kqp", bufs=2) as kqp,
            tc.tile_pool(name="mtp", bufs=2) as mtp,
            tc.tile_pool(name="vaugp", bufs=2) as vaugp,
            tc.tile_pool(name="chunk", bufs=3) as chp,
            tc.tile_pool(name="atm", bufs=3) as atmp,
            tc.tile_pool(name="norm", bufs=2) as normp,
            tc.tile_pool(name="ao", bufs=2) as aop,
            tc.tile_pool(name="psS", bufs=2, space="PSUM") as psS,
            tc.tile_pool(name="psV", bufs=1, space="PSUM") as psV,
            tc.tile_pool(name="psT", bufs=1, space="PSUM") as psTp,
            tc.tile_pool(name="psP", bufs=2, space="PSUM") as psP,
        ):
          pending_tail = []
          for rep in range(reps):
            # ---- resident loads ----
            wq_sb = constp.tile([128, 4, 768], bf16, tag="wqkv", name="wq_sb")
            nc.sync.dma_start(out=wq_sb[:], in_=wqkvh[:])
            bq_sb = constp.tile([128, 2], f32, tag="bq", name="bq_sb")
            nc.sync.dma_start(out=bq_sb[:], in_=bqh[:])
            bk_sb = constp.tile([128, 2], f32, tag="bk", name="bk_sb")
            nc.sync.dma_start(out=bk_sb[:], in_=bkh[:])
            bvb_sb = constp.tile([128, 256], f32, tag="bvb", name="bvb_sb")
            nc.sync.dma_start(out=bvb_sb[:], in_=bvb[:])
            id_sb = constp.tile([128, 128], bf16, tag="ident", name="id_sb")
            nc.sync.dma_start(out=id_sb[:], in_=ident[:])
            xt_sb = constp.tile([128, 4, S], bf16, tag="xt", name="xt_sb")
            for jb in range(4):
                nc.sync.dma_start(out=xt_sb[:, :, jb * 512 : (jb + 1) * 512],
                                  in_=xT[:, :, jb * 512 : (jb + 1) * 512])
            xqt_sb = constp.tile([128, 4, QS], bf16, tag="xqt", name="xqt_sb")
            nc.sync.dma_start(out=xqt_sb[:], in_=xqT[:])
            mt_sb = mtp.tile([128, NJT, QS], bf16, tag="mt")
            for piece in range(4):
                nc.sync.dma_start(
                    out=mt_sb[:, piece * 4 : (piece + 1) * 4, :],
                    in_=mt[:, piece * 4 : (piece + 1) * 4, :],
                )
            wp_sb = constp.tile([128, 2, D], bf16, tag="wp", name="wp_sb")
            nc.sync.dma_start(out=wp_sb[:], in_=wph[:])

            # persistent K^T / Q^T (2 head-pairs stacked on partitions), bf16
            kt2 = kqp.tile([128, 2, S], bf16, tag="kt2", name=f"kt2_{rep}")
            qt2 = kqp.tile([128, 2, QS], bf16, tag="qt2", name=f"qt2_{rep}")

            # ---- projection helpers (emitted interleaved below) ----
            def emit_kproj(kp, jb):
                kps = psP.tile([128, 512], f32, tag="pp",
                               name=f"kps_{rep}_{kp}_{jb}")
                for dtile in range(4):
                    nc.tensor.matmul(
                        kps[:],
                        wq_sb[:, dtile, 256 + kp * 128 : 256 + (kp + 1) * 128],
                        xt_sb[:, dtile, jb * 512 : (jb + 1) * 512],
                        start=(dtile == 0),
                        stop=(dtile == 3),
                    )
                nc.vector.tensor_scalar(
                    kt2[:, kp, jb * 512 : (jb + 1) * 512], kps[:],
                    bk_sb[:, kp : kp + 1], None, op0=ALU.add,
                )

            def emit_qproj(kp, qc):
                qps = psP.tile([128, 512], f32, tag="pp",
                               name=f"qps_{rep}_{kp}_{qc}")
                for dtile in range(4):
                    nc.tensor.matmul(
                        qps[:],
                        wq_sb[:, dtile, kp * 128 : (kp + 1) * 128],
                        xqt_sb[:, dtile, qc * 512 : (qc + 1) * 512],
                        start=(dtile == 0),
                        stop=(dtile == 3),
                    )
                nc.vector.tensor_scalar(
                    qt2[:, kp, qc * 512 : (qc + 1) * 512], qps[:],
                    bq_sb[:, kp : kp + 1], None, op0=ALU.add,
                )

            v_aug = vaugp.tile([128, NJT, HH * (HD + 1)], bf16, tag="vaug",
                               name=f"vaug_{rep}")
            nc.vector.memset(
                v_aug[:, :, :].rearrange("p t (h e) -> p t h e", e=HD + 1)[
                    :, :, :, HD : HD + 1
                ],
                1.0,
            )

            def emit_vproj(jt):
                vps = psP.tile([128, 256], f32, tag="pp",
                               name=f"vps_{rep}_{jt}")
                for dtile in range(4):
                    nc.tensor.matmul(
                        vps[:],
                        xt_sb[:, dtile, jt * 128 : (jt + 1) * 128],
                        wq_sb[:, dtile, 512:768],
                        start=(dtile == 0),
                        stop=(dtile == 3),
                    )
                dst = v_aug[:, jt, :].rearrange("p (h e) -> p h e", e=HD + 1)[
                    :, :, 0:HD
                ]
                nc.vector.tensor_add(
                    dst,
                    vps[:].rearrange("p (h e) -> p h e", e=HD),
                    bvb_sb[:].rearrange("p (h e) -> p h e", e=HD),
                )

            # ---- attention units, software-pipelined ----
            # unit u = (qh, hl). emit order: chunks(u) ... AV(u-1), norm(u-1)
            units = [(qh, hl) for qh in range(2) for hl in range(HH)]
            ao_tiles = {}
            unit_state = {}

            def emit_chunks(u, kt2=kt2, qt2=qt2, mt_sb=mt_sb,
                            unit_state=unit_state, rep=rep):
                qh, hl = units[u]
                kp, hp = hl // 2, hl % 2
                ktv = kt2[64 * hp : 64 * hp + 64, kp, :]
                qtv = qt2[64 * hp : 64 * hp + 64, kp,
                          qh * 512 : (qh + 1) * 512]
                atm = atmp.tile([128, NJT, 512], bf16, tag="atm",
                                name=f"atm_{rep}_{u}")
                for ch in range(NCHUNK):
                    sps = psS.tile([128, 2, 512], f32, tag="sc",
                                   name=f"sps_{rep}_{u}_{ch}")
                    at = chp.tile([128, 2, 512], bf16, tag="at",
                                  name=f"at_{rep}_{u}_{ch}")
                    for jc in range(2):
                        jt = 2 * ch + jc
                        nc.tensor.matmul(
                            sps[:, jc, :],
                            ktv[:, jt * 128 : (jt + 1) * 128],
                            qtv[:],
                            start=True, stop=True,
                        )
                    # prologue interleave: V/K/Q projections ride between the
                    # early units' score chunks. AV runs at pipeline depth 2
                    # for the first units, so v_aug is needed only at AV(0)
                    # (emitted after chunks(2)); K1-jb0/Q1-qc0 before unit 2's
                    # first scores, later K1 j-blocks two chunks ahead of use.
                    if u == 0 and stage >= 2:
                        emit_vproj(ch)
                    if u == 1:
                        if stage >= 2:
                            emit_vproj(8 + ch)
                        if stage >= 1 and ch == 3:
                            emit_kproj(1, 0)
                        if stage >= 1 and ch == 5:
                            emit_qproj(1, 0)
                    if u == 2 and stage >= 1:
                        if ch in (0, 2, 4):
                            emit_kproj(1, 1 + ch // 2)
                        elif ch == 6:
                            emit_qproj(0, 1)
                    if u == 3 and ch == 0 and stage >= 1:
                        emit_qproj(1, 1)
                    nc.scalar.activation(at[:], sps[:], AF.Exp, scale=SCALE)
                    muleng = nc.gpsimd if ch in (2, 5) else nc.vector
                    muleng.tensor_mul(
                        atm[:, 2 * ch : 2 * ch + 2, :], at[:],
                        mt_sb[:, 2 * ch : 2 * ch + 2,
                              qh * 512 : (qh + 1) * 512],
                    )
                unit_state[u] = atm

            def emit_av_norm(u, unit_state=unit_state,
                             ao_tiles=ao_tiles, v_aug=v_aug, rep=rep):
                qh, hl = units[u]
                atm = unit_state[u]
                avps = psV.tile([128, 4, HD + 1], f32, tag="av",
                                name=f"avps_{rep}_{u}")
                for qt in range(4):
                    for jt in range(NJT):
                        nc.tensor.matmul(
                            avps[:, qt, :],
                            atm[:, jt, qt * 128 : (qt + 1) * 128],
                            v_aug[:, jt, hl * (HD + 1) : (hl + 1) * (HD + 1)],
                            start=(jt == 0), stop=(jt == NJT - 1),
                        )
                if stage < 4:
                    return
                ao_sb = ao_tiles[qh]
                rec = normp.tile([128, 4], f32, tag="rec",
                                 name=f"rec_{rep}_{u}")
                nc.vector.reciprocal(
                    rec[:], avps[:, :, HD : HD + 1].rearrange("p q one -> p (q one)"))
                for qt in range(4):
                    nc.vector.tensor_scalar(
                        ao_sb[:, qt, hl * 64 : (hl + 1) * 64],
                        avps[:, qt, 0:HD], rec[:, qt : qt + 1], None, op0=ALU.mult,
                    )

            def emit_epilogue(qh, ao_tiles=ao_tiles, id_sb=id_sb,
                              wp_sb=wp_sb, rep=rep):
                # transpose ao + partial output projection for one seq-half
                if stage < 5:
                    return
                ao_sb = ao_tiles[qh]
                psT = psTp.tile([128, 4, 256], bf16, tag="tr",
                                name=f"psT_{rep}_{qh}")
                for st in range(4):
                    for ddt in range(2):
                        nc.tensor.transpose(
                            psT[:, st, ddt * 128 : (ddt + 1) * 128],
                            ao_sb[:, st, ddt * 128 : (ddt + 1) * 128],
                            id_sb[:],
                        )
                aot = normp.tile([128, 4, 256], bf16, tag="aoT",
                                 name=f"aoT_{rep}_{qh}")
                nc.vector.tensor_copy(aot[:], psT[:])
                for st in range(4):
                    ops = psP.tile([128, 512], f32, tag="pp",
                                   name=f"ops_{rep}_{qh}_{st}")
                    for ddt in range(2):
                        nc.tensor.matmul(
                            ops[:], aot[:, st, ddt * 128 : (ddt + 1) * 128],
                            wp_sb[:, ddt, :],
                            start=(ddt == 0), stop=(ddt == 1),
                        )
                    osb = normp.tile([128, D], bf16, tag="osb",
                                     name=f"osb_{rep}_{qh}_{st}")
                    nc.vector.tensor_copy(osb[:], ops[:])
                    nc.gpsimd.dma_start(
                        out=out[qh * 512 + st * 128 : qh * 512 + (st + 1) * 128, :],
                        in_=osb[:],
                    )

            if stage >= 1:
                for jb in range(4):
                    emit_kproj(0, jb)
                emit_qproj(0, 0)
            if stage >= 3:
                ao_tiles[0] = aop.tile([128, 4, 256], bf16, tag="ao",
                                       name=f"ao0_{rep}")
                ao_tiles[1] = aop.tile([128, 4, 256], bf16, tag="ao",
                                       name=f"ao1_{rep}")
                # deferred tail of the previous rep: its last unit's chunks
                # ride in this rep's head windows (this rep's K0/Q0 were just
                # emitted and execute in the previous unit-6 window's slack)
                if pending_tail:
                    t_ch7, t_av7, t_ep1 = pending_tail.pop()
                else:
                    t_ch7 = t_av7 = t_ep1 = None
                if t_ch7 is not None:
                    t_ch7()
                emit_chunks(0)
                if t_av7 is not None:
                    t_av7()
                emit_chunks(1)
                emit_chunks(2)
                emit_av_norm(0)
                emit_chunks(3)
                emit_av_norm(1)
                if t_ep1 is not None:
                    t_ep1()
                emit_chunks(4)
                emit_av_norm(2)
                emit_av_norm(3)
                emit_chunks(5)
                emit_av_norm(4)
                emit_epilogue(0)
                emit_chunks(6)
                emit_av_norm(5)

                def _t_ch7(c=emit_chunks, a=emit_av_norm):
                    c(7)
                    a(6)

                def _t_av7(a=emit_av_norm):
                    a(7)

                def _t_ep1(e=emit_epilogue):
                    e(1)

                pending_tail.append((_t_ch7, _t_av7, _t_ep1))
          if pending_tail:
              for f in pending_tail.pop():
                  f()

    _split_sync_waits(nc)
    return nc


_NC_CACHE = {}


def _get_nc(reps=1, stage=99):
    if (reps, stage) not in _NC_CACHE:
        _NC_CACHE[(reps, stage)] = _build_nc(reps, stage)
    return _NC_CACHE[(reps, stage)]


# ---------------------------------------------------------------------------
# host wrapper
# ---------------------------------------------------------------------------
def _prep_inputs(x, routes, w_qkv, b_qkv, w_proj, b_proj):
    x = np.asarray(x, dtype=np.float32)
    routes = np.asarray(routes)
    w_qkv = np.asarray(w_qkv, dtype=np.float32)
    b_qkv = np.asarray(b_qkv, dtype=np.float32)
    w_proj = np.asarray(w_proj, dtype=np.float32)
    b_proj = np.asarray(b_proj, dtype=np.float32)
    bf = ml_dtypes.bfloat16

    r = np.clip(routes[:S].astype(np.int64), 0, S - 1)
    # multiplicity mask M[s, j] = count of j in routes[s]
    flat = (np.arange(S, dtype=np.int64)[:, None] * S + r).ravel()
    M = np.bincount(flat, minlength=S * S).reshape(S, S).astype(np.float32)

    def t_layout(w, n_out, ntile):  # w: (n_out, 512) -> (128, ntile, n_out)
        return np.ascontiguousarray(
            w.T.reshape(ntile, 128, n_out).transpose(1, 0, 2).astype(bf)
        )

    id128 = np.eye(128, dtype=np.float32).astype(bf)

    in_maps = []
    for c in range(NCORES):
        b, hh, sh = c // 4, (c % 4) // 2, c % 2
        hd0 = hh * 256
        # weights for this head-half: Q rows [hd0, hd0+256), K rows
        # [D+hd0, ...), V rows [2D+hd0, ...) -> [128, 4, 768] (q|k|v)
        wsel = np.concatenate(
            [w_qkv[hd0 : hd0 + 256],
             w_qkv[D + hd0 : D + hd0 + 256],
             w_qkv[2 * D + hd0 : 2 * D + hd0 + 256]], axis=0
        )  # (768, 512)
        wqkvh = t_layout(wsel, 768, 4)
        # wproj columns for this head-half's ao dims -> [128, 2, 512]
        wph = np.ascontiguousarray(
            w_proj[:, hd0 : hd0 + 256].T.reshape(2, 128, D)
            .transpose(1, 0, 2).astype(bf)
        )
        bqh = np.ascontiguousarray(
            b_qkv[hd0 : hd0 + 256].reshape(2, 128).T.astype(np.float32))
        bkh = np.ascontiguousarray(
            b_qkv[D + hd0 : D + hd0 + 256].reshape(2, 128).T.astype(np.float32))
        bvbh = np.ascontiguousarray(
            np.tile(b_qkv[2 * D + hd0 : 2 * D + hd0 + 256], (128, 1))
        ).astype(np.float32)

        xb = x[b]  # (S, D)
        xTc = np.ascontiguousarray(
            xb.T.reshape(4, 128, S).transpose(1, 0, 2).astype(bf))
        s0 = sh * QS
        xqTc = np.ascontiguousarray(
            xb[s0 : s0 + QS].T.reshape(4, 128, QS).transpose(1, 0, 2).astype(bf)
        )
        # mt[p, t, q] = M[s0+q, t*128+p]
        mtc = M[s0 : s0 + QS].T.reshape(NJT, 128, QS).transpose(1, 0, 2)
        mtc = np.ascontiguousarray(mtc.astype(bf))
        in_maps.append(
            {
                "xT": xTc, "xqT": xqTc, "wqkvh": wqkvh, "wph": wph,
                "bqh": bqh, "bkh": bkh, "bvb": bvbh, "mt": mtc,
                "ident": id128,
            }
        )
    return in_maps


def run_cores(in_maps, reps=1, stage=99, **kwargs):
    nc = _get_nc(reps, stage)
    return run_bass_kernel_spmd(nc, in_maps, list(range(NCORES)), **kwargs)


def kernel(x, routes, w_qkv, b_qkv, w_proj, b_proj):
    b_proj = np.asarray(b_proj, dtype=np.float32)
    in_maps = _prep_inputs(x, routes, w_qkv, b_qkv, w_proj, b_proj)
    res = run_cores(in_maps)
    out = np.empty((B, S, D), dtype=np.float32)
    for b in range(B):
        for sh in range(2):
            s0 = sh * QS
            out[b, s0 : s0 + QS] = (
                res.results[b * 4 + sh]["out"].astype(np.float32)
                + res.results[b * 4 + 2 + sh]["out"].astype(np.float32)
                + b_proj
            )
    return out
